# revision 1
# baseline (speedup 1.0000x reference)
"""Tensor-parallel causal attention kernel for TRN2 (Bass/Tile), v2.

Sharding: 16 heads / 8 cores = 2 heads per core. Each core computes
q,k,v projections for its heads, RoPE, causal attention, and a partial
output projection (row-shard of wo). Host sums the 8 partial outputs.

v2 changes vs baseline:
  - v projection computed directly in natural [tok, dv] layout
    (no PE transposes, no vq/vn copies)
  - softmax row-sums via DVE accumulation chains + one ones-column PE
    matmul reduction and ones-row broadcast per (head, q-tile)
  - causal diagonal tiles trimmed: sc/exp/av restricted to valid columns,
    narrow [128,128] triangle mask
  - weights / qf / kf / vn / e tiles in bf16 (SBUF + removes f32r
    small-free-dim matmul penalty); activations stay f32r
  - interleaved emission P(t) / A(qt) / O(qt) + rebalanced PSUM pools
    (qk 2 | sm 2 | sc 2 | av 2 banks) for cross-phase overlap
"""

from contextlib import ExitStack

import numpy as np

import concourse.bass as bass
import concourse.bass_isa as bass_isa
import concourse.mybir as mybir
import concourse.tile as tile
from concourse import bacc

F32R = mybir.dt.float32r
F32 = mybir.dt.float32
BF16 = mybir.dt.bfloat16
AF = mybir.ActivationFunctionType


def build_nc(B=4, S=2048, DIM=2048, HPC=2, n_cores=8,
             xt_bufs=2, qf_bufs=5, kf_bufs=2, rot_bufs=3, vn_bufs=20,
             e_bufs=8, ea_bufs=6, sa_bufs=3, rep_bufs=3, ot_bufs=4,
             op_bufs=4, qk_ps=2, sm_ps=2, sc_ps=2, av_ps=2, look=3):
    P = 128          # partitions
    HD = 128         # head dim
    QT = 512         # query/token tile (moving free dim)
    KC = DIM // P    # contraction chunks for projections
    SC = S // P      # seq chunks per batch
    NQT = S // QT    # q tiles per (b, h)
    JD = QT // P     # 128-sub-blocks per q tile
    DHC = HPC * HD   # per-core qkv width
    MD8 = DIM // DHC  # model-dim tiles (256 wide) for outproj
    NT = B * S
    scale = 1.0 / float(np.sqrt(HD))

    nc = bacc.Bacc("TRN2", target_bir_lowering=False, debug=False,
                   num_devices=n_cores)
    # x pre-packed on host per 512-token tile: xt[p, (tile*KC+kc)*QT+c]
    #   = x[tile*QT+c, kc*P+p] — one contiguous DMA per tile
    xt = nc.dram_tensor("xt", [P, (NT // 512) * DIM * 4], BF16,
                        kind="ExternalInput").ap()
    # weights pre-packed on host: wq/wk/wv[p, kc*DHC+c] = w[kc*P+p, c];
    # wo[p, h*DIM+c] = wo[h*HD+p, c] — one wide DMA each
    wq = nc.dram_tensor("wq", [P, KC * DHC], BF16, kind="ExternalInput").ap()
    wk = nc.dram_tensor("wk", [P, KC * DHC], BF16, kind="ExternalInput").ap()
    wv = nc.dram_tensor("wv", [P, KC * DHC], BF16, kind="ExternalInput").ap()
    wo = nc.dram_tensor("wo", [P, HPC * DIM], BF16,
                        kind="ExternalInput").ap()
    cc = nc.dram_tensor("cc", [HD, S], F32R, kind="ExternalInput").ap()
    ss = nc.dram_tensor("ss", [HD, S], F32R, kind="ExternalInput").ap()
    trid = nc.dram_tensor("tri", [P, P], BF16, kind="ExternalInput").ap()
    onesd = nc.dram_tensor("ones", [P, P + 1], F32R,
                           kind="ExternalInput").ap()
    out = nc.dram_tensor("out", [NT, DIM], BF16,
                         kind="ExternalOutput").ap()

    with ExitStack() as ctx:
        tc = ctx.enter_context(tile.TileContext(nc))
        wpool = ctx.enter_context(tc.tile_pool(name="weights", bufs=1))
        xpool = ctx.enter_context(tc.tile_pool(name="xtp", bufs=xt_bufs))
        qfp = ctx.enter_context(tc.tile_pool(name="qfp", bufs=qf_bufs))
        kfp = ctx.enter_context(tc.tile_pool(name="kfp", bufs=kf_bufs))
        rotp = ctx.enter_context(tc.tile_pool(name="rot", bufs=rot_bufs))
        rawp = ctx.enter_context(tc.tile_pool(name="raw", bufs=rot_bufs))
        vnp = ctx.enter_context(tc.tile_pool(name="vn", bufs=vn_bufs))
        expp = ctx.enter_context(tc.tile_pool(name="expp", bufs=e_bufs))
        eap = ctx.enter_context(tc.tile_pool(name="eap", bufs=ea_bufs))
        sap = ctx.enter_context(tc.tile_pool(name="sap", bufs=sa_bufs))
        repp = ctx.enter_context(tc.tile_pool(name="rep", bufs=rep_bufs))
        otp = ctx.enter_context(tc.tile_pool(name="ot", bufs=ot_bufs))
        opp = ctx.enter_context(tc.tile_pool(name="op", bufs=op_bufs))
        qkps = ctx.enter_context(tc.tile_pool(name="qkps", bufs=qk_ps,
                                              space="PSUM"))
        smps = ctx.enter_context(tc.tile_pool(name="smps", bufs=sm_ps,
                                              space="PSUM"))
        scps = ctx.enter_context(tc.tile_pool(name="scps", bufs=sc_ps,
                                              space="PSUM"))
        avps = ctx.enter_context(tc.tile_pool(name="avps", bufs=av_ps,
                                              space="PSUM"))

        # ---- persistent constants (one wide DMA per weight) ----
        wq_t = wpool.tile([P, KC * DHC], BF16, tag="wq")
        wk_t = wpool.tile([P, KC * DHC], BF16, tag="wk")
        wv_t = wpool.tile([P, KC * DHC], BF16, tag="wv")
        wo_t = wpool.tile([P, HPC * DIM], BF16, tag="wo")
        cc_t = wpool.tile([HD, S], F32R, tag="cc")
        ss_t = wpool.tile([HD, S], F32R, tag="ss")
        tri_t = wpool.tile([P, P], BF16, tag="tri")
        ones_t = wpool.tile([P, P + 1], F32R, tag="ones")
        # spread preamble loads over idle queues so all weights land early
        nc.scalar.dma_start(wq_t[:], wq[:, :])
        nc.scalar.dma_start(wv_t[:], wv[:, :])
        nc.scalar.dma_start(cc_t[:], cc[:, :])
        nc.scalar.dma_start(ss_t[:], ss[:, :])
        nc.gpsimd.dma_start(wk_t[:], wk[:, :])
        nc.scalar.dma_start(tri_t[:], trid[:, :])
        nc.scalar.dma_start(ones_t[:], onesd[:, :])
        ones_col = ones_t[:, 0:1]
        ones_row = ones_t[0:1, 1:P + 1]
        nc.gpsimd.dma_start(wo_t[:], wo[:, :])

        for b in range(B):
            tok0 = b * S
            kf = [kfp.tile([P, S], BF16, tag=f"kf{h}", name=f"kf{h}")
                  for h in range(HPC)]
            qfs = [[None] * NQT for _ in range(HPC)]
            vns = [None] * SC

            def rope(ps, dest, tsl):
                # single ACT copy frees the PSUM bank; rest runs from SBUF
                raw = rawp.tile([P, QT], F32R, tag="raw", name="raw")
                nc.scalar.copy(raw[:], ps[:])
                rot = rotp.tile([P, QT], F32R, tag="rot", name="rot")
                nc.scalar.copy(rot[0:HD // 2, :], raw[HD // 2:HD, :])
                nc.scalar.copy(rot[HD // 2:HD, :], raw[0:HD // 2, :])
                nc.vector.tensor_mul(rot[:], rot[:], ss_t[:, tsl])
                nc.vector.tensor_mul(dest, raw[:], cc_t[:, tsl])
                nc.vector.tensor_add(dest, dest, rot[:])

            def emit_P(t):
                tsl = slice(t * QT, (t + 1) * QT)
                tile_idx = b * NQT + t
                xtile = xpool.tile([P, KC * QT], BF16, tag="xt", name="xt")
                nc.sync.dma_start(
                    xtile[:],
                    xt[:, tile_idx * KC * QT:(tile_idx + 1) * KC * QT])
                xts = [xtile[:, kc * QT:(kc + 1) * QT] for kc in range(KC)]
                def emit_qk(h, is_q):
                    wt = wq_t if is_q else wk_t
                    ps = qkps.tile([P, QT], F32, tag="qk", name="qk")
                    for kc in range(KC):
                        wsl = wt[:, kc * DHC + h * HD:
                                 kc * DHC + (h + 1) * HD]
                        nc.tensor.matmul(ps[:], wsl, xts[kc],
                                         start=(kc == 0),
                                         stop=(kc == KC - 1))
                    if is_q:
                        qfs[h][t] = qfp.tile([P, QT], BF16, tag=f"qf{h}",
                                             name=f"qf{h}")
                        rope(ps, qfs[h][t][:], tsl)
                    else:
                        rope(ps, kf[h][:, tsl], tsl)

                def emit_v(blk):
                    bsl = slice(blk * P, (blk + 1) * P)
                    vp = smps.tile([P, DHC], F32, tag="sm", name="sm")
                    for kc in range(KC):
                        nc.tensor.matmul(
                            vp[:], xtile[:, kc * QT + blk * P:
                                         kc * QT + (blk + 1) * P],
                            wv_t[:, kc * DHC:(kc + 1) * DHC],
                            start=(kc == 0), stop=(kc == KC - 1))
                    vt = vnp.tile([P, DHC], BF16, tag="vn", name="vn")
                    nc.scalar.copy(vt[:], vp[:])
                    vns[t * JD + blk] = vt

                # interleave v blocks between q/k groups so PE has filler
                # work while RoPE drains each qk PSUM bank
                emit_qk(0, True)
                emit_v(0)
                emit_qk(1, True)
                emit_v(1)
                emit_qk(0, False)
                emit_v(2)
                emit_qk(1, False)
                emit_v(3)

            def emit_AO(qt):
                n_kc = JD * (qt + 1)
                # dual sum-accumulator chains per head: even i on DVE into
                # ea[h][0], odd i on Pool into ea[h][1] (qt>0 only; qt==0 is
                # short and diagonal from i==1, single DVE chain)
                dual = qt > 0
                avs = [avps.tile([P, QT], F32, tag="av", name="av")
                       for _ in range(HPC)]
                ea = [[eap.tile([P, QT], F32R, tag="ea", name="ea")
                       for _ in range(2 if dual else 1)]
                      for _ in range(HPC)]
                ess = [[None] * n_kc for _ in range(HPC)]

                def emit_sc(h, i):
                    j = i - JD * qt
                    lo = j * P if 0 <= j < JD else 0
                    sc = scps.tile([P, QT], F32, tag="sc", name="sc")
                    nc.tensor.matmul(sc[:, lo:QT], kf[h][:, i * P:(i + 1) * P],
                                     qfs[h][qt][:, lo:QT],
                                     start=True, stop=True)
                    e = expp.tile([P, QT], BF16, tag="exp", name="exp")
                    nc.scalar.activation(e[:, lo:QT], sc[:, lo:QT], AF.Exp,
                                         scale=scale)
                    if 0 <= j < JD:
                        nc.vector.tensor_mul(e[:, lo:lo + P], e[:, lo:lo + P],
                                             tri_t[:])
                    if i == 0 or (dual and i == 1):
                        nc.vector.tensor_copy(
                            ea[h][min(i, len(ea[h]) - 1)][:, lo:QT],
                            e[:, lo:QT])
                    elif dual and i % 2 == 1:
                        nc.vector.tensor_add(ea[h][1][:, lo:QT],
                                             ea[h][1][:, lo:QT], e[:, lo:QT])
                    else:
                        nc.vector.tensor_add(ea[h][0][:, lo:QT],
                                             ea[h][0][:, lo:QT], e[:, lo:QT])
                    ess[h][i] = (e, lo)

                def emit_av(h, i):
                    e, lo = ess[h][i]
                    hsl = slice(h * HD, (h + 1) * HD)
                    nc.tensor.matmul(avs[h][:, lo:QT], vns[i][:, hsl],
                                     e[:, lo:QT],
                                     start=(i == 0), stop=(i == n_kc - 1))
                    ess[h][i] = None

                for i in range(n_kc):
                    for h in range(HPC):
                        emit_sc(h, i)
                    if i >= look:
                        for h in range(HPC):
                            emit_av(h, i - look)
                for i in range(max(0, n_kc - look), n_kc):
                    for h in range(HPC):
                        emit_av(h, i)

                ots = []
                for h in range(HPC):
                    if dual:
                        nc.vector.tensor_add(ea[h][0][:], ea[h][0][:],
                                             ea[h][1][:])
                    sm_ps = scps.tile([1, QT], F32, tag="sc", name="sc")
                    nc.tensor.matmul(sm_ps[:], ones_col, ea[h][0][:],
                                     start=True, stop=True)
                    rep = repp.tile([P, QT], F32R, tag="rep", name="rep")
                    with nc.allow_low_precision(reason="f32r is f32-width"):
                        nc.vector.reciprocal(rep[0:1, :], sm_ps[:])
                    rp_ps = scps.tile([P, QT], F32, tag="sc", name="sc")
                    nc.tensor.matmul(rp_ps[:], ones_row, rep[0:1, :],
                                     start=True, stop=True)
                    nc.vector.tensor_copy(rep[:], rp_ps[:])
                    ot = otp.tile([P, QT], BF16, tag="ot", name="ot")
                    nc.vector.tensor_mul(ot[:], avs[h][:], rep[:])
                    ots.append(ot)

                # output projection for this tile of tokens
                for tcl in range(JD):
                    csl = slice(tcl * P, (tcl + 1) * P)
                    r0 = tok0 + qt * QT + tcl * P
                    ost = opp.tile([P, DIM], BF16, tag="op", name="op")
                    for mdt in range(DIM // QT):
                        ops = smps.tile([P, QT], F32, tag="sm", name="sm")
                        for h in range(HPC):
                            nc.tensor.matmul(
                                ops[:], ots[h][:, csl],
                                wo_t[:, h * DIM + mdt * QT:
                                     h * DIM + (mdt + 1) * QT],
                                start=(h == 0), stop=(h == HPC - 1))
                        osl = ost[:, mdt * QT:(mdt + 1) * QT]
                        if mdt % 2 == 0:
                            nc.scalar.copy(osl, ops[:])
                        else:
                            nc.vector.tensor_copy(osl, ops[:])
                    nc.sync.dma_start(out[r0:r0 + P, :], ost[:])

            for t in range(NQT):
                emit_P(t)
                emit_AO(t)
    return nc


def prep_shared(x, cos, sin, QT=512, P=128):
    """Host-side layout prep (transpose/concat only, no FLOPs on x)."""
    import ml_dtypes
    B, S, DIM = x.shape
    c = np.arange(P)[None, :]
    p = np.arange(P)[:, None]
    return dict(
        xt=np.ascontiguousarray(
            x.reshape(B * S // QT, QT, DIM // P, P).transpose(3, 0, 2, 1)
            .reshape(P, -1)).astype(ml_dtypes.bfloat16),
        cc=np.ascontiguousarray(np.concatenate([cos.T, cos.T], axis=0)),
        ss=np.ascontiguousarray(np.concatenate([-sin.T, sin.T], axis=0)),
        tri=(c >= p).astype(ml_dtypes.bfloat16),
        ones=np.concatenate([np.ones((P, 1), np.float32),
                             np.concatenate([np.ones((1, P), np.float32),
                                             np.zeros((P - 1, P), np.float32)],
                                            axis=0)],
                            axis=1),
    )


def shard_weights(wq, wk, wv, wo, core, n_cores=8, head_dim=128):
    import ml_dtypes
    n_heads = wq.shape[1] // head_dim
    hpc = n_heads // n_cores
    dhc = hpc * head_dim
    c0, c1 = core * dhc, (core + 1) * dhc
    bf = ml_dtypes.bfloat16

    def pack_col(w):  # [DIM, dhc] -> [128, KC*dhc]
        dim = w.shape[0]
        kc = dim // 128
        return np.ascontiguousarray(
            w.reshape(kc, 128, dhc).transpose(1, 0, 2).reshape(128, -1)
        ).astype(bf)

    def pack_row(w):  # [dhc, DIM] -> [128, hpc*DIM]
        return np.ascontiguousarray(
            w.reshape(hpc, 128, w.shape[1]).transpose(1, 0, 2).reshape(
                128, -1)).astype(bf)

    return dict(
        wq=pack_col(wq[:, c0:c1]),
        wk=pack_col(wk[:, c0:c1]),
        wv=pack_col(wv[:, c0:c1]),
        wo=pack_row(wo[c0:c1, :]),
    )


# ---------------------------------------------------------------------------
# Self-contained entry point: kernel(**inputs) -> np.ndarray
# ---------------------------------------------------------------------------
import jax
from jax.sharding import Mesh, PartitionSpec
from jax.experimental.shard_map import shard_map

import concourse.bass2jax as bass2jax

N_CORES = 8
_CACHE = {}


def _get_runner():
    if "runner" in _CACHE:
        return _CACHE["runner"]
    nc = build_nc()
    nc.compile()
    bass2jax.install_neuronx_cc_hook()
    partition_name = (nc.partition_id_tensor.name
                      if nc.partition_id_tensor else None)
    in_names, out_names, out_avals, zero_outs = [], [], [], []
    for alloc in nc.m.functions[0].allocations:
        if not isinstance(alloc, mybir.MemoryLocationSet):
            continue
        name = alloc.memorylocations[0].name
        if alloc.kind == "ExternalInput":
            if name != partition_name:
                in_names.append(name)
        elif alloc.kind == "ExternalOutput":
            shape = tuple(alloc.tensor_shape)
            dtype = mybir.dt.np(alloc.dtype)
            out_names.append(name)
            out_avals.append(jax.core.ShapedArray(shape, dtype))
            zero_outs.append(np.zeros(shape, dtype))
    all_in_names = in_names + out_names
    if partition_name is not None:
        all_in_names = all_in_names + [partition_name]

    def _body(*args):
        operands = list(args)
        if partition_name is not None:
            operands.append(bass2jax.partition_id_tensor())
        outs = bass2jax._bass_exec_p.bind(
            *operands,
            out_avals=tuple(out_avals),
            in_names=tuple(all_in_names),
            out_names=tuple(out_names),
            lowering_input_output_aliases=(),
            sim_require_finite=True,
            sim_require_nnan=True,
            nc=nc,
        )
        return tuple(outs)

    devices = jax.devices()[:N_CORES]
    mesh = Mesh(np.asarray(devices), ("core",))
    n_in = len(in_names) + len(out_names)
    sharded = jax.jit(
        shard_map(_body, mesh=mesh,
                  in_specs=(PartitionSpec("core"),) * n_in,
                  out_specs=(PartitionSpec("core"),) * len(out_names),
                  check_rep=False),
        keep_unused=True)
    sharding = jax.sharding.NamedSharding(mesh, PartitionSpec("core"))
    _CACHE["runner"] = (sharded, in_names, out_names, out_avals, zero_outs,
                        sharding)
    return _CACHE["runner"]


def _device_inputs(x, cos, sin, wq, wk, wv, wo):
    shared = prep_shared(np.asarray(x, dtype=np.float32),
                         np.asarray(cos, dtype=np.float32),
                         np.asarray(sin, dtype=np.float32))
    in_maps = []
    for c in range(N_CORES):
        m = dict(shared)
        m.update(shard_weights(np.asarray(wq, dtype=np.float32),
                               np.asarray(wk, dtype=np.float32),
                               np.asarray(wv, dtype=np.float32),
                               np.asarray(wo, dtype=np.float32), c,
                               n_cores=N_CORES))
        in_maps.append(m)
    sharded, in_names, out_names, out_avals, zero_outs, sharding = \
        _get_runner()
    concat_in = [np.concatenate([np.asarray(in_maps[c][n])
                                 for c in range(N_CORES)], axis=0)
                 for n in in_names]
    concat_zero = [np.zeros((N_CORES * z.shape[0], *z.shape[1:]), z.dtype)
                   for z in zero_outs]
    dev_in = [jax.device_put(a, sharding) for a in concat_in + concat_zero]
    for a in dev_in:
        a.block_until_ready()
    return dev_in


def _gather(outs, B, S, DIM):
    full = np.asarray(outs[0]).reshape(N_CORES, B * S, DIM)
    return full.sum(axis=0, dtype=np.float32).reshape(B, S, DIM)


def kernel(x, cos, sin, wq, wk, wv, wo):
    """Full inputs in, full output out; work sharded over 8 NeuronCores."""
    B, S, DIM = x.shape
    dev_in = _device_inputs(x, cos, sin, wq, wk, wv, wo)
    sharded = _get_runner()[0]
    outs = sharded(*dev_in)
    jax.block_until_ready(outs)
    return _gather(outs, B, S, DIM)


def measure_hw_time(x, cos, sin, wq, wk, wv, wo, k_lo=5, k_hi=105, trials=6):
    """Marginal per-call time of pipelined executions (min slope)."""
    import time as _time
    dev_in = _device_inputs(x, cos, sin, wq, wk, wv, wo)
    sharded = _get_runner()[0]
    outs = sharded(*dev_in)
    jax.block_until_ready(outs)

    def timed(k):
        t0 = _time.time()
        rs = None
        for _ in range(k):
            rs = sharded(*dev_in)
        jax.block_until_ready(rs)
        return _time.time() - t0

    slopes = []
    for _ in range(trials):
        t_lo = timed(k_lo)
        t_hi = timed(k_hi)
        slopes.append((t_hi - t_lo) / (k_hi - k_lo))
    return min(slopes)

